# revision 12
# baseline (speedup 1.0000x reference)
"""Chamfer loss on 8 Trainium2 NeuronCores.

Data-parallel over batch B=8: one batch element per core. Per core the
[N, M] = [2048, 2048] squared-distance matrix is produced on the
TensorEngine as K=4 matmuls using the expansion
    d2[i,j] = |x_i|^2 + |y_j|^2 - 2 x_i . y_j
with augmented operands  lhsT = [x0; x1; |x|^2; 1]  (4 x 2048) and
    rhs = [-2 y0; -2 y1; 1; |y|^2]
(prepared host-side, O(N) work). Since sqrt is monotone, row/col minima
are taken over d2 and sqrt is applied to the 2*2048 minima only. The
ScalarEngine drains PSUM to SBUF as bf16; the VectorEngine does a
log2 fold-chain of tensor_tensor(min) for row minima (bf16 SBUF = 2
elem/cycle/lane) plus a running elementwise col-min. Column minima
across partitions are finished with 16 PE transposes and one multi-dim
reduce. Device ships per-partition sums of sqrt(min); host finishes
with a 128-element sum per core and the batch mean.
"""

import numpy as np

B, N, M, D = 8, 2048, 2048, 2
P = 128            # partition tile (X rows per strip)
TN = N // P        # 16 strips
NBLK = 512         # matmul moving free dim (one PSUM bank of fp32)
HBLK = 1024        # PSUM strip-half width (2 banks)
K_AUG = 18         # contraction rows: 6 hi/lo/lolo products per coord + split norms
BIG = 3.0e38

_nc_cache = {}
last_results = None
TRACE = False


def _build(reps=1):
    """reps>1 wraps the whole computation in a hardware For_i loop —
    used only for steady-state timing measurements."""
    import concourse.bacc as bacc
    import concourse.tile as tile
    from concourse import mybir
    from concourse.masks import make_identity
    from contextlib import nullcontext

    f32 = mybir.dt.float32
    bf16 = mybir.dt.bfloat16
    Alu = mybir.AluOpType

    nc = bacc.Bacc(
        "TRN2",
        target_bir_lowering=False,
        debug=False,
        enable_asserts=False,
        num_devices=B,
    )
    lhs_d = nc.dram_tensor("lhs_aug", [K_AUG, N], bf16, kind="ExternalInput")
    rhs_d = nc.dram_tensor("rhs_aug", [K_AUG, M], bf16, kind="ExternalInput")
    out_d = nc.dram_tensor("out", [P, 1], f32, kind="ExternalOutput")

    with tile.TileContext(nc) as tc:
        with (
            tc.tile_pool(name="const", bufs=1) as const,
            tc.tile_pool(name="strips", bufs=4) as strips,
            tc.tile_pool(name="scratch", bufs=2) as scratch_pool,
            tc.tile_pool(name="groups", bufs=2) as groups_pool,
            tc.tile_pool(name="psum_d2", bufs=3, space="PSUM") as pd2,
            tc.tile_pool(name="psum_epi", bufs=1, space="PSUM") as pepi,
        ):
            lhsT = const.tile([K_AUG, N], bf16)
            rhsT = const.tile([K_AUG, M], bf16)
            nc.sync.dma_start(out=lhsT, in_=lhs_d.ap())
            nc.sync.dma_start(out=rhsT, in_=rhs_d.ap())

            ident = const.tile([P, P], bf16)
            make_identity(nc, ident)

            acc = const.tile([P, M], bf16)      # running col-min
            xy = const.tile([P, 2 * TN], f32)   # [:, :TN] row mins, [:, TN:] col mins
            dist = const.tile([P, 2 * TN], f32)
            sums = const.tile([P, 1], f32)

            GRP = 4  # strips per batched row-min tail
            loop_cm = tc.For_i(0, reps, 1) if reps > 1 else nullcontext()
            with loop_cm:
                gbuf = None
                for s in range(TN):
                    bstrip = strips.tile([P, M], bf16, name="bstrip")
                    for h in range(M // HBLK):
                        d2 = pd2.tile([P, HBLK], f32, name="d2")
                        for j in range(HBLK // NBLK):
                            nc.tensor.matmul(
                                d2[:, j * NBLK : (j + 1) * NBLK],
                                lhsT[:, s * P : (s + 1) * P],
                                rhsT[:, h * HBLK + j * NBLK : h * HBLK + (j + 1) * NBLK],
                                start=True,
                                stop=True,
                            )
                        nc.scalar.copy(bstrip[:, h * HBLK : (h + 1) * HBLK], d2)
                    # row-min: per-strip fold 2048->512 at 2 elem/cycle, then
                    # the 512->1 tail is batched over GRP strips to amortize
                    # per-op overhead
                    if s % GRP == 0:
                        gbuf = groups_pool.tile([P, GRP, M // 4], bf16, name="gbuf")
                    fold = scratch_pool.tile([P, M // 2], bf16, name="fold")
                    nc.vector.tensor_tensor(
                        fold, bstrip[:, : M // 2], bstrip[:, M // 2 :], op=Alu.min
                    )
                    nc.vector.tensor_tensor(
                        gbuf[:, s % GRP, :],
                        fold[:, : M // 4],
                        fold[:, M // 4 :],
                        op=Alu.min,
                    )
                    # col-min accumulate (first strip seeds acc via 4x copy)
                    if s == 0:
                        nc.vector.tensor_copy(acc, bstrip)
                    else:
                        nc.vector.tensor_tensor(acc, acc, bstrip, op=Alu.min)
                    if s % GRP == GRP - 1:
                        w = M // 8
                        while w >= P:
                            nc.vector.tensor_tensor(
                                gbuf[:, :, :w],
                                gbuf[:, :, :w],
                                gbuf[:, :, w : 2 * w],
                                op=Alu.min,
                            )
                            w //= 2
                        nc.vector.tensor_reduce(
                            out=xy[:, s - GRP + 1 : s + 1],
                            in_=gbuf[:, :, : 2 * w],
                            axis=mybir.AxisListType.X,
                            op=Alu.min,
                        )

                # partition-min of acc via PE transposes + one multi-dim reduce
                accT = pepi.tile([P, TN, P], bf16, name="accT")
                for t in range(TN):
                    nc.tensor.transpose(
                        accT[:, t, :], acc[:, t * P : (t + 1) * P], ident
                    )
                nc.vector.tensor_reduce(
                    out=xy[:, TN : 2 * TN],
                    in_=accT,
                    axis=mybir.AxisListType.X,
                    op=Alu.min,
                )
                # d2 minima can round slightly negative; clamp before sqrt
                nc.vector.tensor_scalar_max(xy, xy, 0.0)
                nc.scalar.sqrt(dist, xy)
                nc.vector.reduce_sum(sums, dist, axis=mybir.AxisListType.X)
                nc.sync.dma_start(out=out_d.ap(), in_=sums)

    nc.compile()
    return nc


def _split3(v):
    """3-way bf16 split: v ~= h + l + ll with ~2^-27 relative residual."""
    import ml_dtypes

    bf = ml_dtypes.bfloat16
    h = v.astype(bf)
    r = v - h.astype(np.float32)
    l = r.astype(bf)
    ll = (r - l.astype(np.float32)).astype(bf)
    return h, l, ll


def _prep_core(x, y):
    """Host-side per-core operand prep: O(N) layout, norms, bf16 splits.

    Summing lhsT[k]*rhs[k] over the 18 rows reconstructs
    |x|^2 + |y|^2 - 2 x.y with ~2^-27-scale absolute error (products of
    bf16 values are exact in the fp32 PSUM accumulator; only the
    representation residual and the dropped l*ll cross terms remain).
    Per coordinate (w = -2y): h*h', h*l', l*h', l*l', h*ll', ll*h'.
    Norms enter as 3-way splits against ones.
    """
    import ml_dtypes

    bf = ml_dtypes.bfloat16
    x = np.ascontiguousarray(x, dtype=np.float32)
    y = np.ascontiguousarray(y, dtype=np.float32)
    w = -2.0 * y
    nx = (x.astype(np.float64) ** 2).sum(axis=1).astype(np.float32)
    ny = (y.astype(np.float64) ** 2).sum(axis=1).astype(np.float32)

    lhs = np.empty((K_AUG, N), dtype=bf)
    rhs = np.empty((K_AUG, M), dtype=bf)
    k = 0
    for c in range(2):
        xh, xl, xll = _split3(x[:, c])
        wh, wl, wll = _split3(w[:, c])
        for a, b in ((xh, wh), (xh, wl), (xl, wh), (xl, wl), (xh, wll), (xll, wh)):
            lhs[k], rhs[k] = a, b
            k += 1
    one_n = np.ones(N, bf)
    one_m = np.ones(M, bf)
    for part in _split3(nx):
        lhs[k], rhs[k] = part, one_m
        k += 1
    for part in _split3(ny):
        lhs[k], rhs[k] = one_n, part
        k += 1
    assert k == K_AUG
    return {"lhs_aug": lhs, "rhs_aug": rhs}


def run(pds, pred_pds, reps=1, trace=None):
    global last_results
    from concourse import bass_utils

    pds = np.asarray(pds)
    pred_pds = np.asarray(pred_pds)
    assert pds.shape == (B, N, D) and pred_pds.shape == (B, M, D)

    if reps not in _nc_cache:
        _nc_cache[reps] = _build(reps)
    nc = _nc_cache[reps]

    in_maps = [_prep_core(pds[b], pred_pds[b]) for b in range(B)]
    last_results = bass_utils.run_bass_kernel_spmd(
        nc, in_maps, core_ids=list(range(B)),
        trace=TRACE if trace is None else trace,
    )
    vals = [
        float(last_results.results[b]["out"].sum()) / (2.0 * N) for b in range(B)
    ]
    return np.float32(np.mean(vals))


def kernel(pds, pred_pds):
    return run(pds, pred_pds, reps=1)
